# revision 1
# baseline (speedup 1.0000x reference)
"""Trainium2 Bass kernel for nn_CustomMultiresLayer (B=2, D=1024, L=4096, FS=4).

Sharding (8 cores): core c -> batch beta=c//4, channel shard gamma=c%4
(256 channels). Phase A computes the depthwise multires tree + gated
combination for the core's 256 channels. An AllGather within each 4-core
batch group assembles the full gated tensor y; each core then computes
its 256 OUTPUT channels of the 1x1 channel mix (w_mix @ y) over all 4096
positions, adds the residual, computes partial LayerNorm channel sums,
AllReduces the tiny [2,4096] stats, and normalizes its channel slab.

Engine plan per tree level: a-chain convs as diagonal-weight matmuls on
the tensor engine (bf16 in, fp32 PSUM accumulate), b convs split between
PE and DVE, sigmoid on ACT, gating mul/add on DVE. Channel mix: bf16
matmul, fp32 PSUM. LN stats via fp32r ones-matmuls over partitions,
normalization via fp32r outer-product scale/shift matrices.
"""

import numpy as np
import ml_dtypes

import concourse.bacc as bacc
import concourse.mybir as mybir
import concourse.tile as tile
from concourse.bass_utils import run_bass_kernel_spmd

F32 = mybir.dt.float32
F32R = mybir.dt.float32r
BF16 = mybir.dt.bfloat16
AF = mybir.ActivationFunctionType
ALU = mybir.AluOpType

B, D, L = 2, 1024, 4096
FS, DEPTH = 4, 11
LN_EPS = 1e-5
NC = 8
CH = 256          # channels per core (2 half-tiles of 128)
NMM = 512         # matmul moving-dim tile
GROUPS = [[0, 1, 2, 3], [4, 5, 6, 7]]

# which levels run each conv family on PE (rest on DVE)
PE_A_LEVELS = frozenset()
PE_B_LEVELS = frozenset()

_CACHE = {}


def _emit_conv_pe(nc, ps_pool, dst, src, diag, dil, engine):
    """4-tap dilated causal depthwise conv via diagonal-weight matmuls.
    dst, src: SBUF bf16 [128, L]; diag: SBUF bf16 [128, 4*128]."""
    for c0 in range(0, L, NMM):
        taps = []
        for k in (3, 2, 1, 0):
            s = (3 - k) * dil
            lo = max(0, s - c0)
            if lo < NMM:
                taps.append((k, s, lo))
        pp = ps_pool.tile([128, NMM], F32, tag="cps")
        for i, (k, s, lo) in enumerate(taps):
            nc.tensor.matmul(
                pp[:, lo:NMM],
                diag[:, 128 * k : 128 * (k + 1)],
                src[:, c0 + lo - s : c0 + NMM - s],
                start=(i == 0),
                stop=(i == len(taps) - 1),
            )
        if engine == "act":
            nc.scalar.copy(dst[:, c0 : c0 + NMM], pp[:])
        else:
            nc.vector.tensor_copy(dst[:, c0 : c0 + NMM], pp[:])


def _emit_conv_dve(nc, dst, src, h, dil, zb=None):
    """4-tap conv: tap-3 scaled copy on ACT (frees DVE), 3 MACs on DVE."""
    if zb is None:
        nc.vector.tensor_scalar_mul(dst[:], src[:], h[:, 3:4])
    else:
        nc.scalar.activation(
            dst[:], src[:], AF.Identity, bias=zb[:], scale=h[:, 3:4]
        )
    for k in (2, 1, 0):
        s = (3 - k) * dil
        if s < L:
            nc.vector.scalar_tensor_tensor(
                dst[:, s:L], src[:, 0 : L - s], h[:, k : k + 1], dst[:, s:L],
                ALU.mult, ALU.add,
            )


def _build_program(repeats: int = 1, comm: bool = True, do_tree: bool = True, do_pb: bool = True):
    nc = bacc.Bacc("TRN2", target_bir_lowering=False, debug=False, num_devices=NC)

    xs = nc.dram_tensor("xs", [CH, L], F32, kind="ExternalInput").ap()
    h0s = nc.dram_tensor("h0s", [CH, FS], F32, kind="ExternalInput").ap()
    h1s = nc.dram_tensor("h1s", [CH, FS], F32, kind="ExternalInput").ap()
    d0 = nc.dram_tensor("d0", [2, FS, 128, 128], BF16, kind="ExternalInput").ap()
    d1 = nc.dram_tensor("d1", [2, FS, 128, 128], BF16, kind="ExternalInput").ap()
    wTs = nc.dram_tensor("wTs", [D, CH], BF16, kind="ExternalInput").ap()
    bmixs = nc.dram_tensor("bmixs", [CH, 1], F32, kind="ExternalInput").ap()
    gams = nc.dram_tensor("gams", [1, CH], F32, kind="ExternalInput").ap()
    bets = nc.dram_tensor("bets", [1, CH], F32, kind="ExternalInput").ap()
    og = nc.dram_tensor("og", [CH, L], F32, kind="ExternalOutput").ap()

    with tile.TileContext(nc) as tc:
        for _rep in range(repeats):
            with (
                tc.tile_pool(name="dram", bufs=1, space="DRAM") as dram,
                tc.tile_pool(name="smalls", bufs=1) as smalls,
            ):
                y_loc = dram.tile([CH, L], BF16)
                y_gat = dram.tile([D, L], BF16)
                st_loc = dram.tile([2, L], F32)
                st_glb = dram.tile([2, L], F32)

                # ---------------- Phase A: multires tree ----------------
                with (
                    tc.tile_pool(name="tree", bufs=1) as tp,
                    tc.tile_pool(name="cpsum", bufs=6, space="PSUM") as cps,
                ):
                    a_t = [[tp.tile([128, L], F32, tag=f"a{h}{i}", name=f"a{h}{i}") for i in range(2)]
                           for h in range(2)]
                    b_t = [[tp.tile([128, L], F32, tag=f"b{h}{i}", name=f"b{h}{i}") for i in range(2)]
                           for h in range(2)]
                    sig_sh = tp.tile([128, L], F32, tag="sigsh", name="sigsh")
                    sig = [sig_sh, sig_sh]
                    y_t = [tp.tile([128, L], F32, tag=f"y{h}", name=f"y{h}") for h in range(2)]
                    y16 = [tp.tile([128, L], BF16, tag=f"y16{h}", name=f"y16{h}") for h in range(2)]
                    zb = smalls.tile([128, 1], F32, tag="zb", name="zb")
                    nc.vector.memset(zb[:], 0.0)
                    h0c = [smalls.tile([128, FS], F32, tag=f"h0c{h}", name=f"h0c{h}") for h in range(2)]
                    h1c = [smalls.tile([128, FS], F32, tag=f"h1c{h}", name=f"h1c{h}") for h in range(2)]
                    d0c = [smalls.tile([128, FS * 128], BF16, tag=f"d0c{h}", name=f"d0c{h}") for h in range(2)]
                    d1c = [smalls.tile([128, FS * 128], BF16, tag=f"d1c{h}", name=f"d1c{h}") for h in range(2)]

                    with tc.tile_pool(name="stage", bufs=2) as stage:
                        for h in range(2):
                            rs = slice(128 * h, 128 * (h + 1))
                            nc.sync.dma_start(a_t[h][0][:], xs[rs, :])
                            nc.sync.dma_start(h0c[h][:], h0s[rs, :])
                            nc.sync.dma_start(h1c[h][:], h1s[rs, :])
                            for k in range(FS):
                                ks = slice(128 * k, 128 * (k + 1))
                                nc.sync.dma_start(d0c[h][:, ks], d0[h, k])
                                nc.sync.dma_start(d1c[h][:, ks], d1[h, k])

                    for lvl in range(DEPTH if do_tree else 2):
                        dil = 1 << lvl
                        for h in range(2):
                            a_cur = a_t[h][lvl % 2]
                            a_nxt = a_t[h][(lvl + 1) % 2]
                            b_cur = b_t[h][lvl % 2]
                            b_prv = b_t[h][(lvl + 1) % 2]
                            if lvl in PE_A_LEVELS:
                                _emit_conv_pe(nc, cps, a_nxt, a_cur, d0c[h], dil, "act")
                            else:
                                _emit_conv_dve(nc, a_nxt, a_cur, h0c[h], dil, zb)
                            if lvl < DEPTH - 1:
                                if lvl in PE_B_LEVELS:
                                    _emit_conv_pe(nc, cps, b_cur, a_cur, d1c[h], dil, "dve")
                                else:
                                    _emit_conv_dve(nc, b_cur, a_cur, h1c[h], dil, zb)
                            if lvl >= 1:
                                nc.scalar.activation(sig[h][:], a_nxt[:], AF.Sigmoid)
                                nc.gpsimd.tensor_mul(sig[h][:], sig[h][:], b_prv[:])
                                if lvl == 1:
                                    nc.vector.tensor_scalar_mul(y_t[h][:], sig[h][:], 2.0)
                                else:
                                    nc.gpsimd.tensor_add(y_t[h][:], y_t[h][:], sig[h][:])

                    for h in range(2):
                        nc.vector.tensor_copy(y16[h][:], y_t[h][:])
                        nc.sync.dma_start(y_loc[128 * h : 128 * (h + 1), :], y16[h][:])

                if comm:
                    nc.gpsimd.collective_compute(
                        "AllGather",
                        ALU.bypass,
                        replica_groups=GROUPS,
                        ins=[y_loc.opt()],
                        outs=[y_gat.opt()],
                    )
                else:  # timing-only stand-in, same data volume
                    for _g in range(4):
                        nc.sync.dma_start(
                            y_gat[CH * _g : CH * (_g + 1), :], y_loc[:, :]
                        )

                # ---------------- Phase B: channel mix + LayerNorm ----------------
                if not do_pb:
                    with tc.tile_pool(name="skip", bufs=1) as sk:
                        for o in range(2):
                            tt = sk.tile([128, L], F32, tag="sk")
                            nc.sync.dma_start(tt[:], xs[128 * o : 128 * (o + 1), :])
                            nc.sync.dma_start(og[128 * o : 128 * (o + 1), :], tt[:])
                    continue
                with (
                    tc.tile_pool(name="mix", bufs=1) as mx,
                    tc.tile_pool(name="yld", bufs=1) as yld,
                    tc.tile_pool(name="scr", bufs=2) as scr,
                    tc.tile_pool(name="tiny", bufs=2) as tiny,
                ):
                    wsb = mx.tile([128, 8 * CH], BF16, tag="wsb")      # lhsT per k-chunk
                    xsb = mx.tile([128, 2 * L], F32, tag="xsb")        # residual (o-major)
                    zsb = mx.tile([128, 2 * L], F32R, tag="zsb")
                    osb = mx.tile([128, 2 * L], F32, tag="osb")
                    bsc = smalls.tile([128, 2], F32, tag="bsc")
                    grow = smalls.tile([1, CH], F32R, tag="grow")
                    brow = smalls.tile([1, CH], F32R, tag="brow")
                    ones = smalls.tile([128, 1], F32R, tag="ones")
                    one_r = smalls.tile([1, NMM], F32R, tag="oner")
                    eps_t = smalls.tile([1, 1], F32, tag="eps")

                    for k in range(8):
                        nc.sync.dma_start(
                            wsb[:, CH * k : CH * (k + 1)], wTs[128 * k : 128 * (k + 1), :]
                        )
                    for o in range(2):
                        rs = slice(128 * o, 128 * (o + 1))
                        nc.sync.dma_start(xsb[:, L * o : L * (o + 1)], xs[rs, :])
                        nc.sync.dma_start(bsc[:, o : o + 1], bmixs[rs, :])

                    with tc.tile_pool(name="stage2", bufs=2) as stage2:
                        g32 = stage2.tile([1, CH], F32, tag="g32")
                        b32 = stage2.tile([1, CH], F32, tag="b32")
                        o32 = stage2.tile([128, 1], F32, tag="o32")
                        or32 = stage2.tile([1, NMM], F32, tag="or32")
                        nc.sync.dma_start(g32[:], gams[:])
                        nc.sync.dma_start(b32[:], bets[:])
                        nc.vector.tensor_copy(grow[:], g32[:])
                        nc.vector.tensor_copy(brow[:], b32[:])
                        nc.vector.memset(o32[:], 1.0)
                        nc.vector.tensor_copy(ones[:], o32[:])
                        nc.vector.memset(eps_t[:], LN_EPS)
                        nc.vector.memset(or32[:], 1.0)
                        nc.vector.tensor_copy(one_r[:], or32[:])

                    # matmul + residual + partial stats, halves of the position axis
                    with (
                        tc.tile_pool(name="mmps", bufs=4, space="PSUM") as psmm,
                        tc.tile_pool(name="stps", bufs=2, space="PSUM") as psst,
                    ):
                        for ph in range(2):
                            yhs = yld.tile([128, 8 * (L // 2)], BF16, tag="yhs")
                            for k in range(8):
                                nc.sync.dma_start(
                                    yhs[:, (L // 2) * k : (L // 2) * (k + 1)],
                                    y_gat[128 * k : 128 * (k + 1),
                                          (L // 2) * ph : (L // 2) * (ph + 1)],
                                )
                            for nth in range(L // 2 // NMM):   # 4 n-tiles per half
                                n0 = (L // 2) * ph + NMM * nth  # global position offset
                                pms = []
                                for o in range(2):
                                    pm = psmm.tile([128, NMM], F32, tag="mm")
                                    pms.append(pm)
                                    for k in range(8):
                                        nc.tensor.matmul(
                                            pm[:],
                                            wsb[:, CH * k + 128 * o :
                                                CH * k + 128 * (o + 1)],
                                            yhs[:, (L // 2) * k + NMM * nth :
                                                (L // 2) * k + NMM * (nth + 1)],
                                            start=(k == 0),
                                            stop=(k == 7),
                                        )
                                ps_sum = psst.tile([1, NMM], F32, tag="sum")
                                ps_sq = psst.tile([1, NMM], F32, tag="sq")
                                for o in range(2):
                                    zc = slice(L * o + n0, L * o + n0 + NMM)
                                    nc.vector.scalar_tensor_tensor(
                                        zsb[:, zc], pms[o][:], bsc[:, o : o + 1],
                                        xsb[:, zc], ALU.add, ALU.add,
                                    )
                                    nc.tensor.matmul(
                                        ps_sum[:], ones[:], zsb[:, zc],
                                        start=(o == 0), stop=(o == 1),
                                        skip_group_check=True,
                                    )
                                    z2 = scr.tile([128, NMM], F32R, tag="z2")
                                    nc.scalar.square(z2[:], zsb[:, zc])
                                    nc.tensor.matmul(
                                        ps_sq[:], ones[:], z2[:],
                                        start=(o == 0), stop=(o == 1),
                                        skip_group_check=True,
                                    )
                                sc_sum = tiny.tile([1, NMM], F32, tag="scsum")
                                sc_sq = tiny.tile([1, NMM], F32, tag="scsq")
                                nc.vector.tensor_copy(sc_sum[:], ps_sum[:])
                                nc.vector.tensor_copy(sc_sq[:], ps_sq[:])
                                nc.sync.dma_start(
                                    st_loc[0:1, n0 : n0 + NMM], sc_sum[:]
                                )
                                nc.sync.dma_start(
                                    st_loc[1:2, n0 : n0 + NMM], sc_sq[:]
                                )

                    if comm:
                        nc.gpsimd.collective_compute(
                            "AllReduce",
                            ALU.add,
                            replica_groups=GROUPS,
                            ins=[st_loc.opt()],
                            outs=[st_glb.opt()],
                        )
                    else:
                        nc.sync.dma_start(st_glb[:, :], st_loc[:, :])

                    # normalize per 512-position tile: out = z*G + B2
                    with tc.tile_pool(name="gbps", bufs=2, space="PSUM") as psgb:
                        for nt in range(L // NMM):
                            nn = slice(NMM * nt, NMM * (nt + 1))
                            mu = tiny.tile([1, NMM], F32R, tag="mu")
                            e2 = tiny.tile([1, NMM], F32, tag="e2")
                            m2 = tiny.tile([1, NMM], F32, tag="m2")
                            std = tiny.tile([1, NMM], F32, tag="std")
                            inv = tiny.tile([1, NMM], F32R, tag="inv")
                            nms = tiny.tile([1, NMM], F32R, tag="nms")
                            nc.sync.dma_start(m2[:], st_glb[0:1, nn])
                            nc.sync.dma_start(e2[:], st_glb[1:2, nn])
                            nc.vector.tensor_scalar_mul(mu[:], m2[:], 1.0 / D)
                            nc.vector.tensor_scalar_mul(e2[:], e2[:], 1.0 / D)
                            nc.vector.scalar_tensor_tensor(
                                m2[:], mu[:], -1.0, mu[:], ALU.mult, ALU.mult
                            )
                            nc.vector.tensor_add(m2[:], m2[:], e2[:])
                            nc.scalar.activation(std[:], m2[:], AF.Sqrt, bias=eps_t[:])
                            with nc.allow_low_precision(
                                reason="inv_std stored fp32r for PE outer-products"
                            ):
                                nc.vector.reciprocal(inv[:], std[:])
                            nc.vector.scalar_tensor_tensor(
                                nms[:], mu[:], -1.0, inv[:], ALU.mult, ALU.mult
                            )
                            for o in range(2):
                                oc = slice(128 * o, 128 * (o + 1))
                                zc = slice(L * o + NMM * nt, L * o + NMM * (nt + 1))
                                G = psgb.tile([128, NMM], F32, tag="G")
                                B2 = psgb.tile([128, NMM], F32, tag="B2")
                                nc.tensor.matmul(G[:], grow[:, oc], inv[:])
                                nc.tensor.matmul(
                                    B2[:], brow[:, oc], one_r[:],
                                    start=True, stop=False,
                                )
                                nc.tensor.matmul(
                                    B2[:], grow[:, oc], nms[:],
                                    start=False, stop=True,
                                )
                                nc.vector.scalar_tensor_tensor(
                                    osb[:, zc], zsb[:, zc], 1.0, G[:],
                                    ALU.mult, ALU.mult,
                                )
                                nc.vector.scalar_tensor_tensor(
                                    osb[:, zc], osb[:, zc], 1.0, B2[:],
                                    ALU.mult, ALU.add,
                                )
                    for o in range(2):
                        nc.sync.dma_start(
                            og[128 * o : 128 * (o + 1), :],
                            osb[:, L * o : L * (o + 1)],
                        )

    nc.compile()
    return nc


def _get_program(repeats: int = 1, comm: bool = True, do_tree: bool = True, do_pb: bool = True):
    key = f"nc{repeats}_{comm}_{do_tree}_{do_pb}"
    if key not in _CACHE:
        _CACHE[key] = _build_program(repeats, comm, do_tree, do_pb)
    return _CACHE[key]


def _make_in_maps(inputs):
    x = np.ascontiguousarray(np.asarray(inputs["x"], dtype=np.float32))
    h0 = np.asarray(inputs["h0"], dtype=np.float32)[:, 0, :]  # [D, FS]
    h1 = np.asarray(inputs["h1"], dtype=np.float32)[:, 0, :]
    w = np.asarray(inputs["w_mix"], dtype=np.float32)
    bm = np.asarray(inputs["b_mix"], dtype=np.float32).reshape(D, 1)
    gm = np.asarray(inputs["ln_gamma"], dtype=np.float32).reshape(1, D)
    bt = np.asarray(inputs["ln_beta"], dtype=np.float32).reshape(1, D)

    wT16 = np.ascontiguousarray(w.T).astype(ml_dtypes.bfloat16)  # [c, o]

    in_maps = []
    for c in range(NC):
        beta, gamma = c // 4, c % 4
        cs = slice(CH * gamma, CH * (gamma + 1))
        h0c = h0[cs].astype(ml_dtypes.bfloat16)
        h1c = h1[cs].astype(ml_dtypes.bfloat16)
        d0m = np.zeros((2, FS, 128, 128), ml_dtypes.bfloat16)
        d1m = np.zeros((2, FS, 128, 128), ml_dtypes.bfloat16)
        for h in range(2):
            for k in range(FS):
                np.fill_diagonal(d0m[h, k], h0c[128 * h : 128 * (h + 1), k])
                np.fill_diagonal(d1m[h, k], h1c[128 * h : 128 * (h + 1), k])
        in_maps.append(
            {
                "xs": np.ascontiguousarray(x[beta, cs, :]),
                "h0s": np.ascontiguousarray(h0[cs]),
                "h1s": np.ascontiguousarray(h1[cs]),
                "d0": d0m,
                "d1": d1m,
                "wTs": np.ascontiguousarray(wT16[:, cs]),
                "bmixs": np.ascontiguousarray(bm[cs]),
                "gams": np.ascontiguousarray(gm[:, cs]),
                "bets": np.ascontiguousarray(bt[:, cs]),
            }
        )
    return in_maps


def kernel(**inputs) -> np.ndarray:
    in_maps = _make_in_maps(inputs)
    nc = _get_program()
    res = run_bass_kernel_spmd(nc, in_maps, list(range(NC)))

    out = np.empty((B, D, L), dtype=np.float32)
    for c in range(NC):
        beta, gamma = c // 4, c % 4
        out[beta, CH * gamma : CH * (gamma + 1), :] = res.results[c]["og"]
    return out



# revision 10
# speedup vs baseline: 1.6531x; 1.6531x over previous
"""Trainium2 Bass kernel for nn_CustomMultiresLayer (B=2, D=1024, L=4096, FS=4).

Sharding (8 cores): core c -> batch beta=c//4, channel shard gamma=c%4
(256 channels = 2 half-slabs of 128). Phase A computes the depthwise
multires tree + gated combination for the core's 256 channels; the two
128-channel half-slabs are processed sequentially (h=0 fully first) so
the AllGather of half 0 overlaps half 1's tree. Phase B: channel mix as
bf16 matmuls (residual folded in via an identity matmul), partial LN
stats, AllReduce of [2,4096] stats, normalize, store.

Engine split in the tree, per level: a-chain convs on PE (bf16 diagonal
weight matmuls, fp32 PSUM, ACT copyout to bf16), b convs on DVE in bf16
(2x mode), sigmoid on ACT, gate-mul on DVE (bf16 2x), y accumulation on
GPSIMD in fp32.
"""

import numpy as np
import ml_dtypes

import concourse.bacc as bacc
import concourse.mybir as mybir
import concourse.tile as tile
from concourse.bass_utils import run_bass_kernel_spmd

F32 = mybir.dt.float32
BF16 = mybir.dt.bfloat16
AF = mybir.ActivationFunctionType
ALU = mybir.AluOpType

B, D, L = 2, 1024, 4096
FS, DEPTH = 4, 11
LN_EPS = 1e-5
NC = 8
CH = 256          # channels per core (2 half-slabs of 128)
HW = 2048         # PSUM conv window (4 banks)
NMM = 512         # matmul moving-dim tile
GROUPS = [[0, 1, 2, 3], [4, 5, 6, 7]]

_CACHE = {}


def _emit_conv_pe(nc, cps, dst, src, diag):
    """4-tap dilated causal depthwise conv via diagonal-weight matmuls.
    dst, src: SBUF bf16 [128, L]; diag: SBUF bf16 [128, 4*128] with the
    dilation baked into which source columns each tap reads."""
    dil = diag["dil"]
    dg = diag["t"]
    for w in range(L // HW):
        pp = cps.tile([128, HW], F32, tag="cps", name="cps")
        for ci in range(HW // NMM):
            c0 = HW * w + NMM * ci
            taps = []
            for k in (3, 2, 1, 0):
                s = (3 - k) * dil
                lo = max(0, s - c0)
                if lo < NMM:
                    taps.append((k, s, lo))
            for i, (k, s, lo) in enumerate(taps):
                nc.tensor.matmul(
                    pp[:, NMM * ci + lo : NMM * (ci + 1)],
                    dg[:, 128 * k : 128 * (k + 1)],
                    src[:, c0 + lo - s : c0 + NMM - s],
                    start=(i == 0),
                    stop=(i == len(taps) - 1),
                    skip_group_check=True,
                )
        nc.scalar.copy(dst[:, HW * w : HW * (w + 1)], pp[:])


def _emit_conv_dve(nc, dst, src, h, dil):
    """4-tap conv on DVE, all-bf16 (2x modes)."""
    nc.vector.tensor_scalar_mul(dst[:], src[:], h[:, 3:4])
    for k in (2, 1, 0):
        s = (3 - k) * dil
        nc.vector.scalar_tensor_tensor(
            dst[:, s:L], src[:, 0 : L - s], h[:, k : k + 1], dst[:, s:L],
            ALU.mult, ALU.add,
        )


def _build_program():
    nc = bacc.Bacc("TRN2", target_bir_lowering=False, debug=False, num_devices=NC)

    xs = nc.dram_tensor("xs", [CH, L], F32, kind="ExternalInput").ap()
    h1s = nc.dram_tensor("h1s", [CH, FS], F32, kind="ExternalInput").ap()
    d0 = nc.dram_tensor("d0", [2, FS, 128, 128], BF16, kind="ExternalInput").ap()
    d1x2 = nc.dram_tensor("d1x2", [2, FS, 128, 128], BF16, kind="ExternalInput").ap()
    eye = nc.dram_tensor("eye", [128, 128], BF16, kind="ExternalInput").ap()
    wTs = nc.dram_tensor("wTs", [D, CH], BF16, kind="ExternalInput").ap()
    bmixs = nc.dram_tensor("bmixs", [CH, 1], F32, kind="ExternalInput").ap()
    gams = nc.dram_tensor("gams", [1, CH], BF16, kind="ExternalInput").ap()
    bets = nc.dram_tensor("bets", [1, CH], BF16, kind="ExternalInput").ap()
    og = nc.dram_tensor("og", [CH, L], F32, kind="ExternalOutput").ap()

    with tile.TileContext(nc) as tc:
        with (
            tc.tile_pool(name="dram", bufs=1, space="DRAM") as dram,
            tc.tile_pool(name="keep", bufs=1) as keep,
            tc.tile_pool(name="smalls", bufs=1) as smalls,
        ):
            y_loc = [dram.tile([128, L], BF16, name=f"yloc{h}") for h in range(2)]
            y_gat = [dram.tile([512, L], BF16, name=f"ygat{h}") for h in range(2)]
            st_loc = dram.tile([2, L], F32, name="stloc")
            st_glb = dram.tile([2, L], F32, name="stglb")
            st_fin = dram.tile([2, L], BF16, name="stfin")

            # x in bf16 — both a_0 for the tree and the mix residual
            x16 = [keep.tile([128, L], BF16, name=f"x16{h}") for h in range(2)]

            # phase-B constants (loaded early, tiny)
            wsb = keep.tile([128, 8 * CH], BF16, name="wsb")
            eyesb = smalls.tile([128, 128], BF16, name="eyesb")
            bsc = smalls.tile([128, 2], F32, name="bsc")
            grow = smalls.tile([1, CH], BF16, name="grow")
            brow = smalls.tile([1, CH], BF16, name="brow")
            ones16 = smalls.tile([128, 1], BF16, name="ones16")
            one_r = smalls.tile([1, NMM], BF16, name="oner")
            nc.vector.memset(ones16[:], 1.0)
            nc.vector.memset(one_r[:], 1.0)
            nc.sync.dma_start(eyesb[:], eye)
            nc.sync.dma_start(grow[:], gams)
            nc.sync.dma_start(brow[:], bets)
            for k in range(8):
                nc.sync.dma_start(
                    wsb[:, CH * k : CH * (k + 1)], wTs[128 * k : 128 * (k + 1), :]
                )
            for o in range(2):
                nc.sync.dma_start(bsc[:, o : o + 1], bmixs[128 * o : 128 * (o + 1), :])

            # ---------------- Phase A: multires tree ----------------
            h1c = [smalls.tile([128, FS], F32, name=f"h1c{h}") for h in range(2)]
            d0c = [smalls.tile([128, FS * 128], BF16, name=f"d0c{h}") for h in range(2)]
            d1c = [smalls.tile([128, FS * 128], BF16, name=f"d1c{h}") for h in range(2)]
            with tc.tile_pool(name="stage", bufs=2) as stage:
                for h in range(2):
                    nc.sync.dma_start(h1c[h][:], h1s[128 * h : 128 * (h + 1), :])
                    for k in range(FS):
                        ks = slice(128 * k, 128 * (k + 1))
                        nc.sync.dma_start(d0c[h][:, ks], d0[h, k])
                        nc.sync.dma_start(d1c[h][:, ks], d1x2[h, k])
                    xf = stage.tile([128, L], F32, tag="xf", name="xf")
                    nc.sync.dma_start(xf[:], xs[128 * h : 128 * (h + 1), :])
                    nc.vector.tensor_copy(x16[h][:], xf[:])

            with (
                tc.tile_pool(name="tree", bufs=1) as tp,
                tc.tile_pool(name="cpsum", bufs=2, space="PSUM") as cps,
            ):
                for h in range(2):
                    a_t = [tp.tile([128, L], BF16, tag=f"a{h}{i}", name=f"a{h}{i}")
                           for i in range(2)]
                    b_t = [tp.tile([128, L], BF16, tag=f"b{h}{i}", name=f"b{h}{i}")
                           for i in range(2)]
                    sg = tp.tile([128, L], BF16, tag=f"sg{h}", name=f"sg{h}")
                    gt = tp.tile([128, L], BF16, tag=f"gt{h}", name=f"gt{h}")
                    y_t = tp.tile([128, L], F32, tag=f"y{h}", name=f"y{h}")
                    y16 = tp.tile([128, L], BF16, tag=f"y16{h}", name=f"y16{h}")

                    for lvl in range(DEPTH):
                        dil = 1 << lvl
                        a_cur = x16[h] if lvl == 0 else a_t[lvl % 2]
                        a_nxt = a_t[(lvl + 1) % 2]
                        _emit_conv_pe(nc, cps, a_nxt, a_cur,
                                      {"t": d0c[h], "dil": dil})
                        if lvl == 0:
                            # b_0 with doubled h1 (folds the reused last
                            # gated term), on PE to dodge odd-offset DVE
                            _emit_conv_pe(nc, cps, b_t[0], a_cur,
                                          {"t": d1c[h], "dil": 1})
                        elif lvl < DEPTH - 1:
                            _emit_conv_dve(nc, b_t[lvl % 2], a_cur, h1c[h], dil)
                        if lvl >= 1:
                            nc.scalar.activation(sg[:], a_nxt[:], AF.Sigmoid)
                            nc.vector.tensor_mul(gt[:], sg[:], b_t[(lvl + 1) % 2][:])
                            if lvl == 1:
                                nc.gpsimd.tensor_copy(y_t[:], gt[:])
                            else:
                                nc.gpsimd.tensor_add(y_t[:], y_t[:], gt[:])

                    nc.vector.tensor_copy(y16[:], y_t[:])
                    nc.sync.dma_start(y_loc[h][:, :], y16[:])
                    nc.gpsimd.collective_compute(
                        "AllGather",
                        ALU.bypass,
                        replica_groups=GROUPS,
                        ins=[y_loc[h].opt()],
                        outs=[y_gat[h].opt()],
                    )

            # ---------------- Phase B: channel mix + LayerNorm ----------------
            with (
                tc.tile_pool(name="mix", bufs=1) as mx,
                tc.tile_pool(name="yld", bufs=2) as yld,
                tc.tile_pool(name="tiny", bufs=2) as tiny,
            ):
                z16 = mx.tile([128, 2 * L], BF16, name="z16")
                with (
                    tc.tile_pool(name="mmps", bufs=4, space="PSUM") as psmm,
                    tc.tile_pool(name="stps", bufs=2, space="PSUM") as psst,
                ):
                    for ph in range(2):
                        yhs = yld.tile([128, 8 * (L // 2)], BF16, tag="yhs", name="yhs")
                        for kb in range(8):
                            nc.sync.dma_start(
                                yhs[:, (L // 2) * kb : (L // 2) * (kb + 1)],
                                y_gat[kb // 4][128 * (kb % 4) : 128 * (kb % 4 + 1),
                                               (L // 2) * ph : (L // 2) * (ph + 1)],
                            )
                        for nth in range(L // 2 // NMM):
                            n0 = (L // 2) * ph + NMM * nth
                            pms = []
                            for o in range(2):
                                pm = psmm.tile([128, NMM], F32, tag="mm", name="pmm")
                                pms.append(pm)
                                for kb in range(8):
                                    nc.tensor.matmul(
                                        pm[:],
                                        wsb[:, CH * kb + 128 * o :
                                            CH * kb + 128 * (o + 1)],
                                        yhs[:, (L // 2) * kb + NMM * nth :
                                            (L // 2) * kb + NMM * (nth + 1)],
                                        start=(kb == 0),
                                        stop=False,
                                    )
                                nc.tensor.matmul(
                                    pm[:],
                                    eyesb[:],
                                    x16[o][:, n0 : n0 + NMM],
                                    start=False,
                                    stop=True,
                                )
                            ps_sum = psst.tile([1, NMM], F32, tag="sum", name="pssum")
                            ps_sq = psst.tile([1, NMM], F32, tag="sq", name="pssq")
                            for o in range(2):
                                zc = slice(L * o + n0, L * o + n0 + NMM)
                                nc.scalar.activation(
                                    z16[:, zc], pms[o][:], AF.Identity,
                                    bias=bsc[:, o : o + 1],
                                )
                                nc.tensor.matmul(
                                    ps_sum[:], ones16[:], z16[:, zc],
                                    start=(o == 0), stop=(o == 1),
                                    skip_group_check=True,
                                )
                                zq = tiny.tile([128, NMM], BF16, tag="z2", name="z2t")
                                nc.scalar.square(zq[:], z16[:, zc])
                                nc.tensor.matmul(
                                    ps_sq[:], ones16[:], zq[:],
                                    start=(o == 0), stop=(o == 1),
                                    skip_group_check=True,
                                )
                            sc_sum = tiny.tile([1, NMM], F32, tag="scsum", name="scsum")
                            sc_sq = tiny.tile([1, NMM], F32, tag="scsq", name="scsq")
                            nc.vector.tensor_copy(sc_sum[:], ps_sum[:])
                            nc.vector.tensor_copy(sc_sq[:], ps_sq[:])
                            nc.sync.dma_start(st_loc[0:1, n0 : n0 + NMM], sc_sum[:])
                            nc.sync.dma_start(st_loc[1:2, n0 : n0 + NMM], sc_sq[:])

                nc.gpsimd.collective_compute(
                    "AllReduce",
                    ALU.add,
                    replica_groups=GROUPS,
                    ins=[st_loc.opt()],
                    outs=[st_glb.opt()],
                )

                # LN scalar tail on [128, 32] layout (position t = 32p + f)
                with tc.tile_pool(name="lns", bufs=1) as lns:
                    s0 = lns.tile([128, 64], F32, name="s0")
                    mu32 = lns.tile([128, 32], F32, name="mu32")
                    msq = lns.tile([128, 32], F32, name="msq")
                    var32 = lns.tile([128, 32], F32, name="var32")
                    std32 = lns.tile([128, 32], F32, name="std32")
                    inv32 = lns.tile([128, 32], F32, name="inv32")
                    nms32 = lns.tile([128, 32], F32, name="nms32")
                    i16 = lns.tile([128, 32], BF16, name="i16")
                    n16 = lns.tile([128, 32], BF16, name="n16")
                    eps_t = lns.tile([128, 1], F32, name="eps_t")
                    nc.vector.memset(eps_t[:], LN_EPS)
                    stv = st_glb.rearrange("a (p f) -> a p f", p=128)
                    nc.sync.dma_start(s0[:, 0:32], stv[0])
                    nc.sync.dma_start(s0[:, 32:64], stv[1])
                    nc.scalar.mul(mu32[:], s0[:, 0:32], 1.0 / D)
                    nc.scalar.square(msq[:], mu32[:])
                    nc.vector.scalar_tensor_tensor(
                        var32[:], s0[:, 32:64], 1.0 / D, msq[:],
                        ALU.mult, ALU.subtract,
                    )
                    nc.scalar.activation(std32[:], var32[:], AF.Sqrt, bias=eps_t[:])
                    nc.vector.reciprocal_approx_fast(inv32[:], std32[:])
                    nc.vector.scalar_tensor_tensor(
                        nms32[:], mu32[:], -1.0, inv32[:], ALU.mult, ALU.mult
                    )
                    nc.vector.tensor_copy(i16[:], inv32[:])
                    nc.vector.tensor_copy(n16[:], nms32[:])
                    sfv = st_fin.rearrange("a (p f) -> a p f", p=128)
                    nc.sync.dma_start(sfv[0], i16[:])
                    nc.sync.dma_start(sfv[1], n16[:])

                ivec = mx.tile([1, L], BF16, name="ivec")
                nvec = mx.tile([1, L], BF16, name="nvec")
                nc.sync.dma_start(ivec[:], st_fin[0:1, :])
                nc.sync.dma_start(nvec[:], st_fin[1:2, :])

                # normalize: out = z*G + B2 with G/B2 via bf16 outer products
                osb = mx.tile([128, 2 * L], F32, name="osb")
                with tc.tile_pool(name="gbps", bufs=2, space="PSUM") as psgb:
                    for nt in range(L // NMM):
                        nn = slice(NMM * nt, NMM * (nt + 1))
                        for o in range(2):
                            oc = slice(128 * o, 128 * (o + 1))
                            zc = slice(L * o + NMM * nt, L * o + NMM * (nt + 1))
                            G = psgb.tile([128, NMM], F32, tag="G", name="G")
                            B2 = psgb.tile([128, NMM], F32, tag="B2", name="B2")
                            nc.tensor.matmul(G[:], grow[:, oc], ivec[:, nn])
                            nc.tensor.matmul(
                                B2[:], brow[:, oc], one_r[:],
                                start=True, stop=False,
                            )
                            nc.tensor.matmul(
                                B2[:], grow[:, oc], nvec[:, nn],
                                start=False, stop=True,
                            )
                            nc.vector.scalar_tensor_tensor(
                                osb[:, zc], z16[:, zc], 1.0, G[:],
                                ALU.mult, ALU.mult,
                            )
                            nc.vector.scalar_tensor_tensor(
                                osb[:, zc], osb[:, zc], 1.0, B2[:],
                                ALU.mult, ALU.add,
                            )
                for o in range(2):
                    nc.sync.dma_start(
                        og[128 * o : 128 * (o + 1), :],
                        osb[:, L * o : L * (o + 1)],
                    )

    nc.compile()
    return nc


def _get_program():
    key = "v2"
    if key not in _CACHE:
        _CACHE[key] = _build_program()
    return _CACHE[key]


def _make_in_maps(inputs):
    x = np.ascontiguousarray(np.asarray(inputs["x"], dtype=np.float32))
    h0 = np.asarray(inputs["h0"], dtype=np.float32)[:, 0, :]  # [D, FS]
    h1 = np.asarray(inputs["h1"], dtype=np.float32)[:, 0, :]
    w = np.asarray(inputs["w_mix"], dtype=np.float32)
    bm = np.asarray(inputs["b_mix"], dtype=np.float32).reshape(D, 1)
    gm = np.asarray(inputs["ln_gamma"], dtype=np.float32).reshape(1, D)
    bt = np.asarray(inputs["ln_beta"], dtype=np.float32).reshape(1, D)

    wT = np.ascontiguousarray(w.T)  # [c, o]
    # permute input-channel rows to the split-AllGather order:
    # new row r = h*512 + g*128 + j  <->  original channel c = g*256 + h*128 + j
    wTp = (
        wT.reshape(4, 2, 128, D)
        .transpose(1, 0, 2, 3)
        .reshape(D, D)
    )
    eye = np.eye(128, dtype=np.float32).astype(ml_dtypes.bfloat16)

    in_maps = []
    for c in range(NC):
        beta, gamma = c // 4, c % 4
        cs = slice(CH * gamma, CH * (gamma + 1))
        h0c = h0[cs].astype(ml_dtypes.bfloat16)
        h1c2 = (2.0 * h1[cs]).astype(ml_dtypes.bfloat16)
        d0m = np.zeros((2, FS, 128, 128), ml_dtypes.bfloat16)
        d1m = np.zeros((2, FS, 128, 128), ml_dtypes.bfloat16)
        for h in range(2):
            for k in range(FS):
                np.fill_diagonal(d0m[h, k], h0c[128 * h : 128 * (h + 1), k])
                np.fill_diagonal(d1m[h, k], h1c2[128 * h : 128 * (h + 1), k])
        in_maps.append(
            {
                "xs": np.ascontiguousarray(x[beta, cs, :]),
                "h1s": np.ascontiguousarray(h1[cs]),
                "d0": d0m,
                "d1x2": d1m,
                "eye": eye,
                "wTs": np.ascontiguousarray(wTp[:, cs]).astype(ml_dtypes.bfloat16),
                "bmixs": np.ascontiguousarray(bm[cs]),
                "gams": np.ascontiguousarray(gm[:, cs]).astype(ml_dtypes.bfloat16),
                "bets": np.ascontiguousarray(bt[:, cs]).astype(ml_dtypes.bfloat16),
            }
        )
    return in_maps


def kernel(**inputs) -> np.ndarray:
    in_maps = _make_in_maps(inputs)
    nc = _get_program()
    res = run_bass_kernel_spmd(nc, in_maps, list(range(NC)))

    out = np.empty((B, D, L), dtype=np.float32)
    for c in range(NC):
        beta, gamma = c // 4, c % 4
        out[beta, CH * gamma : CH * (gamma + 1), :] = res.results[c]["og"]
    return out


# revision 19
# speedup vs baseline: 1.9023x; 1.1508x over previous
"""Trainium2 Bass kernel for nn_CustomMultiresLayer (B=2, D=1024, L=4096, FS=4).

Sharding (8 cores): core c -> batch beta=c//4, channel shard gamma=c%4
(256 channels = 2 half-slabs of 128). Phase A computes the depthwise
multires tree + gated combination for the core's 256 channels; the two
128-channel half-slabs are processed sequentially (h=0 fully first) so
the AllGather of half 0 overlaps half 1's tree. Phase B: channel mix as
bf16 matmuls (residual folded in via an identity matmul), partial LN
stats, AllReduce of [2,4096] stats, normalize, store.

Engine split in the tree, per level: a-chain convs on PE (bf16 diagonal
weight matmuls, fp32 PSUM, ACT copyout to bf16), b convs on DVE in bf16
(2x mode), sigmoid on ACT, gate-mul on DVE (bf16 2x), y accumulation on
GPSIMD in fp32.
"""

import numpy as np
import ml_dtypes

import concourse.bacc as bacc
import concourse.mybir as mybir
import concourse.tile as tile
from concourse.bass_utils import run_bass_kernel_spmd

F32 = mybir.dt.float32
BF16 = mybir.dt.bfloat16
AF = mybir.ActivationFunctionType
ALU = mybir.AluOpType

B, D, L = 2, 1024, 4096
FS, DEPTH = 4, 11
LN_EPS = 1e-5
NC = 8
CH = 256          # channels per core (2 half-slabs of 128)
HW = 2048         # PSUM conv window (4 banks)
NMM = 512         # matmul moving-dim tile
GROUPS = [[0, 1, 2, 3], [4, 5, 6, 7]]

_CACHE = {}


def _emit_conv_pe(nc, cps, dst, src, diag, hi=L):
    """4-tap dilated causal depthwise conv via diagonal-weight matmuls on
    cols [0, hi). dst, src: SBUF bf16 [128, L]; diag: SBUF bf16
    [128, 4*128]. Tap-major emission within each PSUM window so identical
    stationary weights are consecutive."""
    dil = diag["dil"]
    dg = diag["t"]
    for w0 in range(0, hi, HW):
        pp = cps.tile([128, HW], F32, tag="cps", name="cps")
        nch = (min(hi, w0 + HW) - w0) // NMM
        # per chunk: list of valid taps, to place start/stop flags
        valid = []
        for ci in range(nch):
            c0 = w0 + NMM * ci
            valid.append([k for k in (3, 2, 1, 0)
                          if max(0, (3 - k) * dil - c0) < NMM])
        for k in (3, 2, 1, 0):
            s = (3 - k) * dil
            for ci in range(nch):
                c0 = w0 + NMM * ci
                lo = max(0, s - c0)
                if lo >= NMM:
                    continue
                nc.tensor.matmul(
                    pp[:, NMM * ci + lo : NMM * (ci + 1)],
                    dg[:, 128 * k : 128 * (k + 1)],
                    src[:, c0 + lo - s : c0 + NMM - s],
                    start=(k == 3),
                    stop=(k == valid[ci][-1]),
                    skip_group_check=True,
                )
        nc.scalar.copy(dst[:, w0 : w0 + NMM * nch], pp[:, 0 : NMM * nch])


def _emit_conv_dve_tail(nc, dst, src, h, dil, lo=HW):
    """4-tap conv on DVE STT for cols [lo, L). Shifts s=(3-k)*dil must be
    < lo so no causal clipping is needed (true for b convs, dil<=512).
    Tap 3 initializes dst via (src*(h3+1)) - src (h col 4 holds h3+1),
    which avoids the slow TENSOR_SCALAR path."""
    nc.vector.scalar_tensor_tensor(
        dst[:, lo:L], src[:, lo:L], h[:, 4:5], src[:, lo:L],
        ALU.mult, ALU.subtract,
    )
    for k in (2, 1, 0):
        s = (3 - k) * dil
        nc.vector.scalar_tensor_tensor(
            dst[:, lo:L], src[:, lo - s : L - s], h[:, k : k + 1], dst[:, lo:L],
            ALU.mult, ALU.add,
        )


def _build_program():
    nc = bacc.Bacc("TRN2", target_bir_lowering=False, debug=False, num_devices=NC)

    xs = nc.dram_tensor("xs", [CH, L], F32, kind="ExternalInput").ap()
    h1s = nc.dram_tensor("h1s", [CH, FS + 1], F32, kind="ExternalInput").ap()
    d0 = nc.dram_tensor("d0", [2, FS, 128, 128], BF16, kind="ExternalInput").ap()
    d1x2 = nc.dram_tensor("d1x2", [2, FS, 128, 128], BF16, kind="ExternalInput").ap()
    d1p = nc.dram_tensor("d1p", [2, FS, 128, 128], BF16, kind="ExternalInput").ap()
    eye = nc.dram_tensor("eye", [128, 128], BF16, kind="ExternalInput").ap()
    wTs = nc.dram_tensor("wTs", [D, CH], BF16, kind="ExternalInput").ap()
    bmixs = nc.dram_tensor("bmixs", [CH, 1], F32, kind="ExternalInput").ap()
    gams = nc.dram_tensor("gams", [1, CH], BF16, kind="ExternalInput").ap()
    bets = nc.dram_tensor("bets", [1, CH], BF16, kind="ExternalInput").ap()
    og = nc.dram_tensor("og", [CH, L], F32, kind="ExternalOutput").ap()

    with tile.TileContext(nc) as tc:
        with (
            tc.tile_pool(name="dram", bufs=1, space="DRAM") as dram,
            tc.tile_pool(name="keep", bufs=1) as keep,
            tc.tile_pool(name="smalls", bufs=1) as smalls,
        ):
            y_loc = [dram.tile([128, L], BF16, name=f"yloc{h}") for h in range(2)]
            y_gat = [dram.tile([512, L], BF16, name=f"ygat{h}") for h in range(2)]
            st_loc = dram.tile([2, L], F32, name="stloc")
            st_glb = dram.tile([2, L], F32, name="stglb")
            st_fin = dram.tile([2, L], BF16, name="stfin")

            # x in bf16 — both a_0 for the tree and the mix residual
            x16 = [keep.tile([128, L], BF16, name=f"x16{h}") for h in range(2)]

            # phase-B constants (loaded early, tiny)
            wsb = keep.tile([128, 8 * CH], BF16, name="wsb")
            eyesb = smalls.tile([128, 128], BF16, name="eyesb")
            bsc = smalls.tile([128, 2], F32, name="bsc")
            grow = smalls.tile([1, CH], BF16, name="grow")
            brow = smalls.tile([1, CH], BF16, name="brow")
            ones16 = smalls.tile([128, 1], BF16, name="ones16")
            one_r = smalls.tile([1, NMM], BF16, name="oner")
            nc.vector.memset(ones16[:], 1.0)
            nc.vector.memset(one_r[:], 1.0)
            nc.sync.dma_start(eyesb[:], eye)
            nc.sync.dma_start(grow[:], gams)
            nc.sync.dma_start(brow[:], bets)
            for k in range(8):
                nc.sync.dma_start(
                    wsb[:, CH * k : CH * (k + 1)], wTs[128 * k : 128 * (k + 1), :]
                )
            for o in range(2):
                nc.sync.dma_start(bsc[:, o : o + 1], bmixs[128 * o : 128 * (o + 1), :])

            # ---------------- Phase A: multires tree ----------------
            h1c = [smalls.tile([128, FS + 1], F32, name=f"h1c{h}") for h in range(2)]
            d0c = [smalls.tile([128, FS * 128], BF16, name=f"d0c{h}") for h in range(2)]
            d1c = [smalls.tile([128, FS * 128], BF16, name=f"d1c{h}") for h in range(2)]
            d1h = [smalls.tile([128, FS * 128], BF16, name=f"d1h{h}") for h in range(2)]
            with tc.tile_pool(name="stage", bufs=2) as stage:
                for h in range(2):
                    nc.sync.dma_start(h1c[h][:], h1s[128 * h : 128 * (h + 1), :])
                    for k in range(FS):
                        ks = slice(128 * k, 128 * (k + 1))
                        nc.sync.dma_start(d0c[h][:, ks], d0[h, k])
                        nc.sync.dma_start(d1c[h][:, ks], d1x2[h, k])
                        nc.sync.dma_start(d1h[h][:, ks], d1p[h, k])
                    xf = stage.tile([128, L], F32, tag="xf", name="xf")
                    nc.sync.dma_start(xf[:], xs[128 * h : 128 * (h + 1), :])
                    nc.vector.tensor_copy(x16[h][:], xf[:])

            with (
                tc.tile_pool(name="tree", bufs=1) as tp,
                tc.tile_pool(name="cpsum", bufs=2, space="PSUM") as cps,
            ):
                for h in range(2):
                    a_t = [tp.tile([128, L], BF16, tag=f"a{h}{i}", name=f"a{h}{i}")
                           for i in range(2)]
                    b_t = [tp.tile([128, L], BF16, tag=f"b{h}{i}", name=f"b{h}{i}")
                           for i in range(2)]
                    sg = tp.tile([128, L], BF16, tag=f"sg{h}", name=f"sg{h}")
                    gt = tp.tile([128, L], BF16, tag=f"gt{h}", name=f"gt{h}")
                    y_t = tp.tile([128, L], F32, tag=f"y{h}", name=f"y{h}")
                    y16 = tp.tile([128, L], BF16, tag=f"y16{h}", name=f"y16{h}")

                    for lvl in range(DEPTH):
                        dil = 1 << lvl
                        a_cur = x16[h] if lvl == 0 else a_t[lvl % 2]
                        a_nxt = a_t[(lvl + 1) % 2]
                        _emit_conv_pe(nc, cps, a_nxt, a_cur,
                                      {"t": d0c[h], "dil": dil})
                        if lvl == 0:
                            # b_0 with doubled h1 (folds the reused last
                            # gated term), on PE to dodge odd-offset DVE
                            _emit_conv_pe(nc, cps, b_t[0], a_cur,
                                          {"t": d1c[h], "dil": 1})
                        elif lvl < DEPTH - 1:
                            # cols [0,HW) on PE, tail [HW,L) on DVE
                            _emit_conv_pe(nc, cps, b_t[lvl % 2], a_cur,
                                          {"t": d1h[h], "dil": dil}, hi=HW)
                            _emit_conv_dve_tail(nc, b_t[lvl % 2], a_cur,
                                                h1c[h], dil)
                        if lvl >= 1:
                            nc.scalar.activation(sg[:], a_nxt[:], AF.Sigmoid)
                            nc.vector.tensor_mul(gt[:], sg[:], b_t[(lvl + 1) % 2][:])
                            if lvl == 1:
                                nc.vector.tensor_copy(y_t[:], gt[:])
                            else:
                                nc.gpsimd.tensor_add(y_t[:], y_t[:], gt[:])

                    nc.vector.tensor_copy(y16[:], y_t[:])
                    nc.sync.dma_start(y_loc[h][:, :], y16[:])
                    nc.gpsimd.collective_compute(
                        "AllGather",
                        ALU.bypass,
                        replica_groups=GROUPS,
                        ins=[y_loc[h].opt()],
                        outs=[y_gat[h].opt()],
                    )

            # ---------------- Phase B: channel mix + LayerNorm ----------------
            with (
                tc.tile_pool(name="mix", bufs=1) as mx,
                tc.tile_pool(name="yld", bufs=2) as yld,
                tc.tile_pool(name="tiny", bufs=2) as tiny,
            ):
                z16 = mx.tile([128, 2 * L], BF16, name="z16")
                with (
                    tc.tile_pool(name="mmps", bufs=4, space="PSUM") as psmm,
                    tc.tile_pool(name="stps", bufs=2, space="PSUM") as psst,
                ):
                    for ph in range(2):
                        yhs = yld.tile([128, 8 * (L // 2)], BF16, tag="yhs", name="yhs")
                        for kb in range(8):
                            nc.sync.dma_start(
                                yhs[:, (L // 2) * kb : (L // 2) * (kb + 1)],
                                y_gat[kb // 4][128 * (kb % 4) : 128 * (kb % 4 + 1),
                                               (L // 2) * ph : (L // 2) * (ph + 1)],
                            )
                        for nth in range(L // 2 // NMM):
                            n0 = (L // 2) * ph + NMM * nth
                            pms = []
                            for o in range(2):
                                pm = psmm.tile([128, NMM], F32, tag="mm", name="pmm")
                                pms.append(pm)
                                for kb in range(8):
                                    nc.tensor.matmul(
                                        pm[:],
                                        wsb[:, CH * kb + 128 * o :
                                            CH * kb + 128 * (o + 1)],
                                        yhs[:, (L // 2) * kb + NMM * nth :
                                            (L // 2) * kb + NMM * (nth + 1)],
                                        start=(kb == 0),
                                        stop=False,
                                    )
                                nc.tensor.matmul(
                                    pm[:],
                                    eyesb[:],
                                    x16[o][:, n0 : n0 + NMM],
                                    start=False,
                                    stop=True,
                                )
                            ps_sum = psst.tile([1, NMM], F32, tag="sum", name="pssum")
                            ps_sq = psst.tile([1, NMM], F32, tag="sq", name="pssq")
                            for o in range(2):
                                zc = slice(L * o + n0, L * o + n0 + NMM)
                                nc.scalar.activation(
                                    z16[:, zc], pms[o][:], AF.Identity,
                                    bias=bsc[:, o : o + 1],
                                )
                                nc.tensor.matmul(
                                    ps_sum[:], ones16[:], z16[:, zc],
                                    start=(o == 0), stop=(o == 1),
                                    skip_group_check=True,
                                )
                                zq = tiny.tile([128, NMM], BF16, tag="z2", name="z2t")
                                nc.scalar.square(zq[:], z16[:, zc])
                                nc.tensor.matmul(
                                    ps_sq[:], ones16[:], zq[:],
                                    start=(o == 0), stop=(o == 1),
                                    skip_group_check=True,
                                )
                            sc_sum = tiny.tile([1, NMM], F32, tag="scsum", name="scsum")
                            sc_sq = tiny.tile([1, NMM], F32, tag="scsq", name="scsq")
                            nc.scalar.copy(sc_sum[:], ps_sum[:])
                            nc.scalar.copy(sc_sq[:], ps_sq[:])
                            nc.sync.dma_start(st_loc[0:1, n0 : n0 + NMM], sc_sum[:])
                            nc.sync.dma_start(st_loc[1:2, n0 : n0 + NMM], sc_sq[:])

                nc.gpsimd.collective_compute(
                    "AllReduce",
                    ALU.add,
                    replica_groups=GROUPS,
                    ins=[st_loc.opt()],
                    outs=[st_glb.opt()],
                )

                # LN scalar tail on [128, 32] layout (position t = 32p + f)
                with tc.tile_pool(name="lns", bufs=1) as lns:
                    s0 = lns.tile([128, 64], F32, name="s0")
                    mu32 = lns.tile([128, 32], F32, name="mu32")
                    msq = lns.tile([128, 32], F32, name="msq")
                    var32 = lns.tile([128, 32], F32, name="var32")
                    std32 = lns.tile([128, 32], F32, name="std32")
                    inv32 = lns.tile([128, 32], F32, name="inv32")
                    nms32 = lns.tile([128, 32], F32, name="nms32")
                    i16 = lns.tile([128, 32], BF16, name="i16")
                    n16 = lns.tile([128, 32], BF16, name="n16")
                    eps_t = lns.tile([128, 1], F32, name="eps_t")
                    nc.vector.memset(eps_t[:], LN_EPS)
                    stv = st_glb.rearrange("a (p f) -> a p f", p=128)
                    nc.sync.dma_start(s0[:, 0:32], stv[0])
                    nc.sync.dma_start(s0[:, 32:64], stv[1])
                    nc.scalar.mul(mu32[:], s0[:, 0:32], 1.0 / D)
                    nc.scalar.square(msq[:], mu32[:])
                    nc.vector.scalar_tensor_tensor(
                        var32[:], s0[:, 32:64], 1.0 / D, msq[:],
                        ALU.mult, ALU.subtract,
                    )
                    nc.scalar.activation(std32[:], var32[:], AF.Sqrt, bias=eps_t[:])
                    nc.vector.reciprocal_approx_fast(inv32[:], std32[:])
                    nc.vector.scalar_tensor_tensor(
                        nms32[:], mu32[:], -1.0, inv32[:], ALU.mult, ALU.mult
                    )
                    nc.vector.tensor_copy(i16[:], inv32[:])
                    nc.vector.tensor_copy(n16[:], nms32[:])
                    sfv = st_fin.rearrange("a (p f) -> a p f", p=128)
                    nc.sync.dma_start(sfv[0], i16[:])
                    nc.sync.dma_start(sfv[1], n16[:])

                ivec = mx.tile([1, L], BF16, name="ivec")
                nvec = mx.tile([1, L], BF16, name="nvec")
                nc.sync.dma_start(ivec[:], st_fin[0:1, :])
                nc.sync.dma_start(nvec[:], st_fin[1:2, :])

                # normalize: out = z*G + B2 with G/B2 via bf16 outer products
                osb = mx.tile([128, 2 * L], F32, name="osb")
                with tc.tile_pool(name="gbps", bufs=2, space="PSUM") as psgb:
                    for nt in range(L // NMM):
                        nn = slice(NMM * nt, NMM * (nt + 1))
                        for o in range(2):
                            oc = slice(128 * o, 128 * (o + 1))
                            zc = slice(L * o + NMM * nt, L * o + NMM * (nt + 1))
                            G = psgb.tile([128, NMM], F32, tag="G", name="G")
                            B2 = psgb.tile([128, NMM], F32, tag="B2", name="B2")
                            nc.tensor.matmul(G[:], grow[:, oc], ivec[:, nn])
                            nc.tensor.matmul(
                                B2[:], brow[:, oc], one_r[:],
                                start=True, stop=False,
                            )
                            nc.tensor.matmul(
                                B2[:], grow[:, oc], nvec[:, nn],
                                start=False, stop=True,
                            )
                            nc.vector.scalar_tensor_tensor(
                                osb[:, zc], z16[:, zc], 1.0, G[:],
                                ALU.mult, ALU.mult,
                            )
                            nc.vector.scalar_tensor_tensor(
                                osb[:, zc], osb[:, zc], 1.0, B2[:],
                                ALU.mult, ALU.add,
                            )
                for o in range(2):
                    nc.sync.dma_start(
                        og[128 * o : 128 * (o + 1), :],
                        osb[:, L * o : L * (o + 1)],
                    )

    nc.compile()
    return nc


def _get_program():
    key = "v3"
    if key not in _CACHE:
        _CACHE[key] = _build_program()
    return _CACHE[key]


def _make_in_maps(inputs):
    x = np.ascontiguousarray(np.asarray(inputs["x"], dtype=np.float32))
    h0 = np.asarray(inputs["h0"], dtype=np.float32)[:, 0, :]  # [D, FS]
    h1 = np.asarray(inputs["h1"], dtype=np.float32)[:, 0, :]
    w = np.asarray(inputs["w_mix"], dtype=np.float32)
    bm = np.asarray(inputs["b_mix"], dtype=np.float32).reshape(D, 1)
    gm = np.asarray(inputs["ln_gamma"], dtype=np.float32).reshape(1, D)
    bt = np.asarray(inputs["ln_beta"], dtype=np.float32).reshape(1, D)

    wT = np.ascontiguousarray(w.T)  # [c, o]
    # permute input-channel rows to the split-AllGather order:
    # new row r = h*512 + g*128 + j  <->  original channel c = g*256 + h*128 + j
    wTp = (
        wT.reshape(4, 2, 128, D)
        .transpose(1, 0, 2, 3)
        .reshape(D, D)
    )
    eye = np.eye(128, dtype=np.float32).astype(ml_dtypes.bfloat16)

    in_maps = []
    for c in range(NC):
        beta, gamma = c // 4, c % 4
        cs = slice(CH * gamma, CH * (gamma + 1))
        h0c = h0[cs].astype(ml_dtypes.bfloat16)
        h1c2 = (2.0 * h1[cs]).astype(ml_dtypes.bfloat16)
        h1cp = h1[cs].astype(ml_dtypes.bfloat16)
        d0m = np.zeros((2, FS, 128, 128), ml_dtypes.bfloat16)
        d1m = np.zeros((2, FS, 128, 128), ml_dtypes.bfloat16)
        d1pm = np.zeros((2, FS, 128, 128), ml_dtypes.bfloat16)
        for h in range(2):
            for k in range(FS):
                np.fill_diagonal(d0m[h, k], h0c[128 * h : 128 * (h + 1), k])
                np.fill_diagonal(d1m[h, k], h1c2[128 * h : 128 * (h + 1), k])
                np.fill_diagonal(d1pm[h, k], h1cp[128 * h : 128 * (h + 1), k])
        h1ext = np.concatenate([h1[cs], h1[cs][:, 3:4] + 1.0], axis=1)
        in_maps.append(
            {
                "xs": np.ascontiguousarray(x[beta, cs, :]),
                "h1s": np.ascontiguousarray(h1ext),
                "d0": d0m,
                "d1x2": d1m,
                "d1p": d1pm,
                "eye": eye,
                "wTs": np.ascontiguousarray(wTp[:, cs]).astype(ml_dtypes.bfloat16),
                "bmixs": np.ascontiguousarray(bm[cs]),
                "gams": np.ascontiguousarray(gm[:, cs]).astype(ml_dtypes.bfloat16),
                "bets": np.ascontiguousarray(bt[:, cs]).astype(ml_dtypes.bfloat16),
            }
        )
    return in_maps


def kernel(**inputs) -> np.ndarray:
    in_maps = _make_in_maps(inputs)
    nc = _get_program()
    res = run_bass_kernel_spmd(nc, in_maps, list(range(NC)))

    out = np.empty((B, D, L), dtype=np.float32)
    for c in range(NC):
        beta, gamma = c // 4, c % 4
        out[beta, CH * gamma : CH * (gamma + 1), :] = res.results[c]["og"]
    return out
